# revision 1
# baseline (speedup 1.0000x reference)
"""3D Haar wavelet transform (2x2x2, causal temporal pad) on 8 Trainium2 cores.

Input  x: (2, 3, 33, 512, 512) fp32
Output y: (2, 24, 17, 256, 256) fp32   (channel = 3*s + c, s = subband)

Sharding: pure data parallel over H — core ci handles input rows
[64*ci, 64*ci+64) i.e. output rows [32*ci, 32*ci+32).

The host pre-gathers each core's input into layout x'[b, c, i, h, T', w]
(i = temporal offset in the 2-frame pair, T' = output frame; causal pad
baked in by clamping), so every DMA is fully contiguous on both sides
with ~17KB descriptor rows.

Per-core pipeline, per (b, c, T'-chunk of 9|8):
  1 DMA in  -> A[128, G*512]   (partition p = i*64 + h, free = (T', w))
  per T': DVE sum/diff of adjacent w pairs (W-axis Haar stage)
          -> B[128, (sw, w')] in float32r
          one 128x128 matmul (T+H Haar stages + partition permutation,
          fixed +-1 weights; out partition m = di*64 + dj*32 + q)
          DVE/ACT evacuate PSUM -> C[128, (sw, T', w')] with x0.3536
  1 DMA out -> y'[b, c, m, sw, T'chunk, w']  (fully contiguous rows)
Host reorders y' -> y (subband-major channels, h' concat).
"""

import numpy as np

import concourse.bacc as bacc
import concourse.mybir as mybir
from concourse import tile
from concourse.bass_utils import run_bass_kernel_spmd

P = 128
B_, C_, T_, H_, W_ = 2, 3, 33, 512, 512
NCORES = 8
HC = H_ // NCORES          # 64 input rows per core
TP = (T_ + 1) // 2         # 17 output frames
HP = HC // 2               # 32 output rows per core
WP = W_ // 2               # 256 output cols
SCALE = float(np.float32(0.3536))
F32 = mybir.dt.float32
# float32r: TF32-like single-pass matmul (4x faster PE than fp32's 2-pass,
# ~1e-4 rel err). Set False for bit-accurate fp32 (2-pass, slower).
USE_FP32R = True
# T' chunks per (b, c): finer macro-steps -> better DMA/compute overlap
T_CHUNKS = [(0, 9), (9, 8)]


def _haar_matrix() -> np.ndarray:
    """W[p, m]: p = i*64 + h (h = 2q+j), m = di*64 + dj*32 + q, val (-1)^(i*di+j*dj)."""
    W = np.zeros((P, P), dtype=np.float32)
    for i in range(2):
        for h in range(HC):
            j = h & 1
            q = h >> 1
            for di in range(2):
                for dj in range(2):
                    m = di * 64 + dj * 32 + q
                    W[i * 64 + h, m] = (-1.0) ** (i * di + j * dj)
    return W


def build_nc():
    nc = bacc.Bacc("TRN2", target_bir_lowering=False, debug=False)
    # x': [b, c, i, h, T', w] host-pretransposed, pad baked in
    x_d = nc.dram_tensor("x", [B_, C_, 2, HC, TP, W_], F32, kind="ExternalInput")
    # y': [b, c, m, sw, T', w']  (m = m4*32+q output partition)
    y_d = nc.dram_tensor("y", [B_, C_, P, 2, TP, WP], F32, kind="ExternalOutput")
    w_d = nc.inline_tensor(_haar_matrix(), name="haar_w")

    mm_dt = mybir.dt.float32r if USE_FP32R else F32

    with tile.TileContext(nc) as tc:
        with (
            tc.tile_pool(name="wpool", bufs=1) as wpool,
            tc.tile_pool(name="apool", bufs=3) as apool,
            tc.tile_pool(name="bpool", bufs=4) as bpool,
            tc.tile_pool(name="stage", bufs=3) as stage_pool,
            tc.tile_pool(name="psum", bufs=8, space="PSUM") as psum_pool,
        ):
            w_sb = wpool.tile([P, P], mm_dt)
            (nc.gpsimd if USE_FP32R else nc.sync).dma_start(
                out=w_sb[:], in_=w_d[:]
            )

            step = 0
            for b in range(B_):
                for c in range(C_):
                    xin = x_d[b, c].rearrange("i h T w -> (i h) (T w)")
                    yout = y_d[b, c].rearrange("m sw T w -> m sw (T w)")
                    for t0, G in T_CHUNKS:
                        subs = [(s0, min(3, G - s0)) for s0 in range(0, G, 3)]
                        a = apool.tile([P, G * W_], F32, tag="a")
                        for s0, sg in subs:
                            nc.sync.dma_start(
                                out=a[:, s0 * W_ : (s0 + sg) * W_],
                                in_=xin[:, (t0 + s0) * W_ : (t0 + s0 + sg) * W_],
                            )
                        cbig = stage_pool.tile([P, 2 * G * WP], F32, tag="c")
                        cview = cbig.rearrange("p (sw f) -> p sw f", sw=2)
                        for tg in range(G):
                            # W-axis stage: sum/diff of adjacent w pairs
                            av = a[:, tg * W_ : (tg + 1) * W_].rearrange(
                                "p (w k) -> p k w", k=2
                            )
                            bt = bpool.tile([P, W_], mm_dt)
                            weng = nc.vector
                            weng.tensor_add(
                                out=bt[:, 0:WP], in0=av[:, 0], in1=av[:, 1]
                            )
                            weng.tensor_sub(
                                out=bt[:, WP:W_], in0=av[:, 0], in1=av[:, 1]
                            )
                            # T+H stages as one matmul
                            ps = psum_pool.tile([P, W_], F32)
                            nc.tensor.matmul(
                                ps[:], w_sb[:], bt[:], start=True, stop=True
                            )
                            # evacuate + scale into (sw, T'rel, w') staging
                            for sw in range(2):
                                dst = cbig[
                                    :, sw * G * WP + tg * WP : sw * G * WP + (tg + 1) * WP
                                ]
                                src = ps[:, sw * WP : (sw + 1) * WP]
                                if tg % 8 == 7:
                                    nc.vector.tensor_scalar_mul(dst, src, SCALE)
                                else:
                                    nc.scalar.mul(dst, src, SCALE)
                            # out sub-DMA as soon as a 3-T' subgroup is done
                            if tg + 1 in [s0 + sg for s0, sg in subs]:
                                s0 = [s for s, sg in subs if s + sg == tg + 1][0]
                                sg = tg + 1 - s0
                                dst = yout[:, :, (t0 + s0) * WP : (t0 + s0 + sg) * WP]
                                src = cview[:, :, s0 * WP : (s0 + sg) * WP]
                                eng = nc.sync if step % 2 == 0 else nc.scalar
                                eng.dma_start(out=dst, in_=src)
                                step += 1
    nc.compile()
    return nc


_NC_CACHE = None


def _get_nc():
    global _NC_CACHE
    if _NC_CACHE is None:
        _NC_CACHE = build_nc()
    return _NC_CACHE


# xp[tp] = x[max(tp-1, 0)] (causal pad); pair (T', i) reads xp[2T'+i]
_TIDX = np.maximum(np.arange(2 * TP) - 1, 0)


def _prep_core_input(x: np.ndarray, ci: int) -> np.ndarray:
    xc = x[:, :, _TIDX, HC * ci : HC * (ci + 1), :]      # [2,3,34,64,512]
    xc = xc.reshape(B_, C_, TP, 2, HC, W_)               # [b,c,T',i,h,w]
    return np.ascontiguousarray(xc.transpose(0, 1, 3, 4, 2, 5))  # [b,c,i,h,T',w]


def kernel(x: np.ndarray) -> np.ndarray:
    assert x.shape == (B_, C_, T_, H_, W_), x.shape
    x = np.ascontiguousarray(x, dtype=np.float32)
    nc = _get_nc()
    in_maps = [{"x": _prep_core_input(x, ci)} for ci in range(NCORES)]
    res = run_bass_kernel_spmd(nc, in_maps, core_ids=list(range(NCORES)))
    y = np.empty((B_, 8 * C_, TP, H_ // 2, WP), dtype=np.float32)
    for ci in range(NCORES):
        yc = res.results[ci]["y"]                        # [b,c,128,2,17,256]
        yc = yc.reshape(B_, C_, 4, HP, 2, TP, WP)        # m = m4*32+q
        yc = yc.transpose(0, 2, 4, 1, 5, 3, 6)           # [b,m4,sw,c,T',q,w']
        yc = yc.reshape(B_, 8 * C_, TP, HP, WP)          # ch = (2*m4+sw)*3+c
        y[:, :, :, HP * ci : HP * (ci + 1), :] = yc
    return y



# revision 2
# speedup vs baseline: 1.1400x; 1.1400x over previous
"""3D Haar wavelet transform (2x2x2, causal temporal pad) on 8 Trainium2 cores.

v4: fp16 I/O, all-matmul transform, frame-0 dedup, pipelined head/tail.

Input  x: (2, 3, 33, 512, 512) fp32
Output y: (2, 24, 17, 256, 256) fp32   (channel = 3*s + c, s = subband)

Sharding: data parallel over H — core ci handles input rows
[64*ci, 64*ci+64) i.e. output rows [32*ci, 32*ci+32).

Main stream (T' >= 1, no causal-pad duplication): partition
p = i*64 + j*32 + k*16 + r, free f = (bc, T'-1, qh, qwh) = 49152 cols.
One 128x128 stationary matrix W[(i,j,k,r),(di,dj,dk,r)] =
(-1)^(i*di+j*dj+k*dk) does all three Haar stages in a single matmul.

T'=0: both temporal taps read x[0], so di=1 subbands are exactly zero
(host writes constant zeros) and di=0 subbands = 2*Haar2D(x[0]),
computed on-device from a packed two-bc-per-block [128, 1536] tile.

DMA plan: ins on the sync HWDGE ring, outs on the scalar ring (strict —
mixing directions on one ring causes sequencer head-of-line blocking).
The first two chunks are small (1024/2048 cols) and issued via gpsimd
(SWDGE), which can start during the ~5us Tile preamble while the
HWDGE engines are still barriered. Tail outs are split across both
rings since the in-stream is drained by then.
"""

import numpy as np

import concourse.bacc as bacc
import concourse.mybir as mybir
from concourse import tile
from concourse.bass_utils import run_bass_kernel_spmd

P = 128
B_, C_, T_, H_, W_ = 2, 3, 33, 512, 512
NCORES = 8
HC = H_ // NCORES          # 64 input rows per core
TP = (T_ + 1) // 2         # 17 output frames
HP = HC // 2               # 32 output rows per core
WP = W_ // 2               # 256 output cols
SCALE = float(np.float32(0.3536))
F16 = mybir.dt.float16
F32 = mybir.dt.float32

BC = B_ * C_               # 6
COLS = BC * (TP - 1) * 512  # 49152 main-stream cols
GRP = 1024                 # evac group = 2 PSUM banks
MM = 512                   # matmul free size = 1 PSUM bank
T0COLS = (BC // 2) * 512   # 1536
# ramped head (SWDGE, starts during preamble), then steady 3072
CHUNKS = [2048, 2048] + [4096] * 11
assert sum(CHUNKS) == COLS


def _haar_w() -> np.ndarray:
    """W[p, m]: p=(i,j,k,r), m=(di,dj,dk,r), val (-1)^(i*di+j*dj+k*dk)."""
    W = np.zeros((P, P), dtype=np.float16)
    for p in range(P):
        i, j, k, r = p >> 6 & 1, p >> 5 & 1, p >> 4 & 1, p & 15
        for m in range(P):
            di, dj, dk, r2 = m >> 6 & 1, m >> 5 & 1, m >> 4 & 1, m & 15
            if r == r2:
                W[p, m] = (-1.0) ** (i * di + j * dj + k * dk)
    return W


def _haar_w0() -> np.ndarray:
    """W0[p, m]: p=(h,j,k,r), m=(h,dj,dk,r), val (-1)^(j*dj+k*dk)."""
    W = np.zeros((P, P), dtype=np.float16)
    for p in range(P):
        h, j, k, r = p >> 6 & 1, p >> 5 & 1, p >> 4 & 1, p & 15
        for m in range(P):
            h2, dj, dk, r2 = m >> 6 & 1, m >> 5 & 1, m >> 4 & 1, m & 15
            if r == r2 and h == h2:
                W[p, m] = (-1.0) ** (j * dj + k * dk)
    return W


def build_nc():
    nc = bacc.Bacc("TRN2", target_bir_lowering=False, debug=False)
    x_d = nc.dram_tensor("x", [P, COLS], F16, kind="ExternalInput")
    x0_d = nc.dram_tensor("x0", [P, T0COLS], F16, kind="ExternalInput")
    y_d = nc.dram_tensor("y", [P, COLS], F16, kind="ExternalOutput")
    y0_d = nc.dram_tensor("y0", [P, T0COLS], F16, kind="ExternalOutput")
    w_d = nc.inline_tensor(_haar_w(), name="haar_w")
    w0_d = nc.inline_tensor(_haar_w0(), name="haar_w0")

    with tile.TileContext(nc) as tc:
        with (
            tc.tile_pool(name="wpool", bufs=1) as wpool,
            tc.tile_pool(name="apool", bufs=14) as apool,
            tc.tile_pool(name="opool", bufs=6) as opool,
            tc.tile_pool(name="psum", bufs=4, space="PSUM") as psum_pool,
        ):
            w_sb = wpool.tile([P, P], F16, tag="w")
            w0_sb = wpool.tile([P, P], F16, tag="w0")
            nc.gpsimd.dma_start(out=w_sb[:], in_=w_d[:])
            nc.gpsimd.dma_start(out=w0_sb[:], in_=w0_d[:])

            g_total = 0
            c0 = 0
            nch = len(CHUNKS)
            for ch, sz in enumerate(CHUNKS):
                ein = nc.sync
                a = apool.tile([P, sz], F16, tag="a")
                ein.dma_start(out=a[:], in_=x_d[:, c0 : c0 + sz])
                o = opool.tile([P, sz], F16, tag="o")
                for g in range(sz // GRP):
                    ps = psum_pool.tile([P, GRP], F32, tag="ps")
                    for mi in range(GRP // MM):
                        off = g * GRP + mi * MM
                        nc.tensor.matmul(
                            ps[:, mi * MM : (mi + 1) * MM],
                            w_sb[:],
                            a[:, off : off + MM],
                            start=True,
                            stop=True,
                        )
                    dst = o[:, g * GRP : (g + 1) * GRP]
                    if g_total % 3 != 1:
                        nc.vector.tensor_scalar_mul(dst, ps[:], SCALE)
                    else:
                        nc.scalar.mul(dst, ps[:], SCALE)
                    g_total += 1
                if ch == nch - 1:
                    # tail: split across both rings (in-stream is drained)
                    h = sz // 2
                    nc.scalar.dma_start(out=y_d[:, c0 : c0 + h], in_=o[:, :h])
                    nc.sync.dma_start(out=y_d[:, c0 + h : c0 + sz], in_=o[:, h:])
                else:
                    nc.scalar.dma_start(out=y_d[:, c0 : c0 + sz], in_=o[:])
                c0 += sz

            # T'=0 block: 1536 cols, di=0 subbands only, scale 2*SCALE
            a0 = apool.tile([P, T0COLS], F16, tag="a")
            nc.sync.dma_start(out=a0[:], in_=x0_d[:])
            o0 = opool.tile([P, T0COLS], F16, tag="o")
            psA = psum_pool.tile([P, GRP], F32, tag="ps")
            for mi in range(2):
                nc.tensor.matmul(
                    psA[:, mi * MM : (mi + 1) * MM],
                    w0_sb[:],
                    a0[:, mi * MM : (mi + 1) * MM],
                    start=True,
                    stop=True,
                )
            nc.vector.tensor_scalar_mul(o0[:, :GRP], psA[:], 2.0 * SCALE)
            psB = psum_pool.tile([P, GRP], F32, tag="ps")
            nc.tensor.matmul(
                psB[:, :MM], w0_sb[:], a0[:, GRP:T0COLS], start=True, stop=True
            )
            nc.scalar.mul(o0[:, GRP:T0COLS], psB[:, :MM], 2.0 * SCALE)
            nc.scalar.dma_start(out=y0_d[:, :GRP], in_=o0[:, :GRP])
            nc.sync.dma_start(out=y0_d[:, GRP:T0COLS], in_=o0[:, GRP:T0COLS])
    nc.compile()
    return nc


_NC_CACHE = None


def _get_nc():
    global _NC_CACHE
    if _NC_CACHE is None:
        _NC_CACHE = build_nc()
    return _NC_CACHE


def _prep_core_input(x16: np.ndarray, ci: int):
    """Main stream [128, 49152] + T0 stream [128, 1536] for core ci."""
    xm = x16[:, :, 1:, HC * ci : HC * (ci + 1), :]       # [2,3,32,64,512]
    # b c T'' i (qh j) (qwh r k):  h = 2qh + j, w = 32 qwh + 2r + k
    xm = xm.reshape(B_, C_, TP - 1, 2, HP, 2, 16, 16, 2)
    xm = xm.transpose(3, 5, 8, 7, 0, 1, 2, 4, 6)         # i j k r b c T'' qh qwh
    xm = np.ascontiguousarray(xm).reshape(P, COLS)

    x0 = x16[:, :, 0, HC * ci : HC * (ci + 1), :]        # [2,3,64,512]
    x0 = x0.reshape(BC // 2, 2, HP, 2, 16, 16, 2)        # pair half qh j qwh r k
    x0 = x0.transpose(1, 3, 6, 5, 0, 2, 4)               # half j k r pair qh qwh
    x0 = np.ascontiguousarray(x0).reshape(P, T0COLS)
    return xm, x0


def kernel(x: np.ndarray) -> np.ndarray:
    assert x.shape == (B_, C_, T_, H_, W_), x.shape
    x16 = np.asarray(x).astype(np.float16)
    nc = _get_nc()
    in_maps = []
    for ci in range(NCORES):
        xm, x0 = _prep_core_input(x16, ci)
        in_maps.append({"x": xm, "x0": x0})
    res = run_bass_kernel_spmd(nc, in_maps, core_ids=list(range(NCORES)))
    y = np.empty((B_, 8 * C_, TP, H_ // 2, WP), dtype=np.float32)
    y[:, 4 * C_ :, 0, :, :] = 0.0                        # di=1 subbands at T'=0
    for ci in range(NCORES):
        yc = res.results[ci]["y"]                        # [128, 49152] fp16
        t = yc.reshape(2, 2, 2, 16, B_, C_, TP - 1, HP, 16)
        t = t.transpose(4, 0, 1, 2, 5, 6, 7, 8, 3)       # b di dj dk c T'' qh qwh r
        t = t.reshape(B_, 8 * C_, TP - 1, HP, WP)        # ch = s*3+c, qw = qwh*16+r
        y[:, :, 1:, HP * ci : HP * (ci + 1), :] = t.astype(np.float32)

        y0c = res.results[ci]["y0"]                      # [128, 1536] fp16
        t0 = y0c.reshape(2, 2, 2, 16, BC // 2, HP, 16)   # half dj dk r pair qh qwh
        t0 = t0.transpose(4, 0, 1, 2, 5, 6, 3)           # pair half dj dk qh qwh r
        t0 = t0.reshape(B_, C_, 4, HP, WP)               # bc = 2*pair+half -> b,c
        t0 = t0.transpose(0, 2, 1, 3, 4).reshape(B_, 4 * C_, HP, WP)
        y[:, : 4 * C_, 0, HP * ci : HP * (ci + 1), :] = t0.astype(np.float32)
    return y
